# revision 5
# baseline (speedup 1.0000x reference)
"""Trainium2 Bass kernel for nn_AttentionTSSA.

Contract: kernel(**inputs) takes FULL unsharded inputs (numpy), returns the
FULL output. Internally shards batch 16 -> 8 cores x 2 batches, runs one SPMD
Bass program on NeuronCores 0-7 via run_bass_kernel_spmd, and concatenates.

Pipeline (per core, per batch, "transposed" layout [hd, token]):
  x (f32, DRAM) --cast-DMA--> x_bf (bf16, DRAM) --xbar-transpose-DMA--> xT sbuf
  MM1: wT[hd, n] = qkv_wT.T @ xT           (bf16 PE, fp32 psum)
  norm2[hd] = sum_n wT^2                   (DVE tensor_tensor_reduce accum)
  scale[hd] = temp[head]/max(norm2, 1e-24)
  wws = (wT*scale)*wT                      (DVE scalar_tensor_tensor)
  s[h, n] = sum_hd sel[hd,h]*wws           (PE selector matmul, col-packed psum)
  E = exp(s)                               (ACT, no max-subtract: s ~ d/N << 1)
  Z[n] = sum_h E                           (PE ones matmul)
  rz_bc[:, n] = 1/Z[n] broadcast           (DVE recip + PE ones outer product)
  E_bc = B.T @ E  (head->hd broadcast)     (PE selector matmul)
  Pi_bc = E_bc * rz_bc, sigPi_bc = sum_n   (DVE tensor_tensor_reduce)
  dots_pre[hd] = sum_n Pi_bc*wT*wT         (DVE, 2 passes)
  negattn = -1/(1 + dots_pre/(sigPi+1e-8)) (DVE tiny)
  outT = (Pi_bc*negattn)*wT  (in placeover Pi_bc)
  MM2: y[tok, j] = outT.T @ out_wT + b     (bf16 PE, bias via K=1 ones matmul)

Works around a walrus limit (1 sync-wait per instruction) by splitting
multi-wait instructions onto inserted InstNoOp hosts.
"""

import sys

if "/opt/trn_rl_repo" not in sys.path:
    sys.path.insert(0, "/opt/trn_rl_repo")

import numpy as np
import ml_dtypes

import concourse.bass as bass
import concourse.mybir as mybir
import concourse.tile as tile
from concourse.bass_utils import run_bass_kernel_spmd

N_CORES = 8
B, N, DIM, H = 16, 4096, 768, 12
D = DIM // H          # 64
BPC = B // N_CORES    # batches per core = 2
KT = DIM // 128       # 6 k-tiles
CH = 512              # token chunk for matmuls
NCH = N // CH         # 8 chunks
TOKT = N // 128       # 32 token tiles for MM2

F32 = mybir.dt.float32
BF16 = mybir.dt.bfloat16
MULT = mybir.AluOpType.mult
ADD = mybir.AluOpType.add
AF = mybir.ActivationFunctionType

BF_NP = ml_dtypes.bfloat16


def split_multi_waits(nc, max_per_inst=1):
    """Walrus in this container rejects >1 sync wait per instruction; host
    extra waits on InstNoOp instructions inserted just before."""
    ctr = 0
    for f in nc.m.functions:
        for b in f.blocks:
            new_list, changed = [], False
            for i in b.instructions:
                si = i.sync_info
                waits = list(si.on_wait) if si and si.on_wait else []
                if len(waits) > max_per_inst:
                    extras = waits[:-max_per_inst]
                    for w in extras:
                        d = mybir.InstNoOp(name=f"waitsplit-{ctr}", ins=[], outs=[])
                        ctr += 1
                        d.engine = i.engine
                        d.sync_info = mybir.SyncInfo(on_wait=[w], on_update=[])
                        new_list.append(d)
                    si.on_wait = waits[-max_per_inst:]
                    changed = True
                new_list.append(i)
            if changed:
                b.instructions = new_list
    return ctr


def _consts():
    head = np.arange(DIM) // D  # head index per hd column
    selT = np.zeros((128, KT, H), dtype=BF_NP)
    Bsel = np.zeros((H, KT, 128), dtype=BF_NP)
    for c in range(KT):
        for p in range(128):
            h = head[c * 128 + p]
            selT[p, c, h] = 1.0
            Bsel[h, c, p] = 1.0
    ones12 = np.ones((H, 1), dtype=BF_NP)
    ones128_bf = np.ones((1, 128), dtype=BF_NP)
    ones128_f32 = np.ones((1, 128), dtype=np.float32)
    return selT, Bsel, ones12, ones128_bf, ones128_f32


def build_program(split_waits=True):
    nc = bass.Bass("TRN2", target_bir_lowering=False, debug=False)

    x_d = nc.dram_tensor("x", [BPC, N, DIM], F32, kind="ExternalInput")
    qkv_d = nc.dram_tensor("qkv_w", [DIM, DIM], F32, kind="ExternalInput")
    temp_d = nc.dram_tensor("temp", [H, 1], F32, kind="ExternalInput")
    ow_d = nc.dram_tensor("out_w", [DIM, DIM], F32, kind="ExternalInput")
    ob_d = nc.dram_tensor("out_b", [DIM], F32, kind="ExternalInput")
    y_d = nc.dram_tensor("out", [BPC, N, DIM], F32, kind="ExternalOutput")

    selT_np, Bsel_np, ones12_np, ones128bf_np, ones128f_np = _consts()
    selT_c = nc.inline_tensor(selT_np, "selT_c")
    Bsel_c = nc.inline_tensor(Bsel_np, "Bsel_c")
    ones12_c = nc.inline_tensor(ones12_np, "ones12_c")
    ones128bf_c = nc.inline_tensor(ones128bf_np, "ones128bf_c")
    ones128f_c = nc.inline_tensor(ones128f_np, "ones128f_c")

    with tile.TileContext(nc) as tc:
        with (
            tc.tile_pool(name="consts", bufs=1) as consts,
            tc.tile_pool(name="dram", bufs=1, space="DRAM") as dram,
            tc.tile_pool(name="big", bufs=6) as big,        # xT then Pibc/outT
            tc.tile_pool(name="wt", bufs=6) as wtp,
            tc.tile_pool(name="wws", bufs=4) as wwsp,
            tc.tile_pool(name="scr", bufs=2) as scr,
            tc.tile_pool(name="rz", bufs=1) as rzp,
            tc.tile_pool(name="ep", bufs=1) as ep,
            tc.tile_pool(name="nvec", bufs=10) as nvec,
            tc.tile_pool(name="rzvec", bufs=1) as rzvec,
            tc.tile_pool(name="stage", bufs=4) as stage,
            tc.tile_pool(name="ps", bufs=4, space="PSUM") as ps,
            tc.tile_pool(name="ps2", bufs=2, space="PSUM") as ps2,
        ):
            # ---- preamble: consts into SBUF ----
            selT = consts.tile([128, KT, H], BF16, tag="selT")
            nc.sync.dma_start(selT[:], selT_c[:])
            Bsel = consts.tile([H, KT, 128], BF16, tag="Bsel")
            nc.sync.dma_start(Bsel[:], Bsel_c[:])
            ones12 = consts.tile([H, 1], BF16, tag="ones12")
            nc.sync.dma_start(ones12[:], ones12_c[:])
            ones128bf = consts.tile([1, 128], BF16, tag="ones128bf")
            nc.sync.dma_start(ones128bf[:], ones128bf_c[:])
            ones128f = consts.tile([1, 128], F32, tag="ones128f")
            nc.sync.dma_start(ones128f[:], ones128f_c[:])

            outb_bf = consts.tile([1, DIM], BF16, tag="outb")
            nc.gpsimd.dma_start(
                outb_bf[:], ob_d.ap().rearrange("(a b) -> a b", a=1)
            )
            tempbf = consts.tile([H, 1], BF16, tag="tempbf")
            nc.gpsimd.dma_start(tempbf[:], temp_d[:])

            # weights: cast to bf16 in DRAM, then xbar-transpose into SBUF
            qkv_bf = dram.tile([DIM, DIM], BF16, tag="qkv_bf")
            nc.gpsimd.dma_start(qkv_bf[:], qkv_d[:])
            ow_bf = dram.tile([DIM, DIM], BF16, tag="ow_bf")
            nc.gpsimd.dma_start(ow_bf[:], ow_d[:])

            qkv_wT = consts.tile([128, KT, DIM], BF16, tag="qkv_wT")
            ow_wT = consts.tile([128, KT, DIM], BF16, tag="ow_wT")
            for k in range(KT):
                nc.sync.dma_start(
                    qkv_wT[:, k, :], qkv_bf[:, k * 128 : (k + 1) * 128],
                    transpose=True,
                )
                nc.sync.dma_start(
                    ow_wT[:, k, :], ow_bf[:, k * 128 : (k + 1) * 128],
                    transpose=True,
                )

            # temp_bc[:, c] = temp[head(hd)] for tile c (PE broadcast)
            temp_bc = consts.tile([128, KT], F32, tag="temp_bc")
            for c in range(KT):
                pst = ps.tile([128, 512], F32, tag="ps")
                nc.tensor.matmul(
                    pst[:, 0:1], Bsel[:, c, :], tempbf[:], start=True, stop=True
                )
                nc.vector.tensor_copy(temp_bc[:, c : c + 1], pst[:, 0:1])

            # x: cast both batches to bf16 DRAM up front (overlaps compute)
            x_bf = dram.tile([BPC, N, DIM], BF16, tag="x_bf")
            for b in range(BPC):
                nc.gpsimd.dma_start(x_bf[b], x_d[b])

            # ---- per-batch pipeline ----
            for b in range(BPC):
                # Phase A: xT tiles via xbar transpose (DRAM bf16 -> SBUF)
                xT = []
                for k in range(KT):
                    t = big.tile([128, N], BF16, tag="big")
                    nc.sync.dma_start(
                        t[:], x_bf[b, :, k * 128 : (k + 1) * 128], transpose=True
                    )
                    xT.append(t)

                # Phase B1: MM1 -> wT tiles; norm2/scale per tile
                wT = []
                scales = []
                for c in range(KT):
                    wT_c = wtp.tile([128, N], BF16, tag="wt")
                    for j in range(NCH):
                        pst = ps.tile([128, 512], F32, tag="ps")
                        for k in range(KT):
                            nc.tensor.matmul(
                                pst[:],
                                qkv_wT[:, k, c * 128 : (c + 1) * 128],
                                xT[k][:, j * CH : (j + 1) * CH],
                                start=(k == 0),
                                stop=(k == KT - 1),
                            )
                        nc.scalar.copy(wT_c[:, j * CH : (j + 1) * CH], pst[:])
                    wT.append(wT_c)

                    junk = scr.tile([128, N], BF16, tag="scr")
                    norm2 = nvec.tile([128, 1], F32, tag="nvec")
                    nc.vector.scalar_tensor_tensor(
                        junk[:], wT_c[:], 1.0, wT_c[:], MULT, MULT, norm2[:]
                    )
                    n2m = nvec.tile([128, 1], F32, tag="nvec")
                    nc.vector.tensor_scalar_max(n2m[:], norm2[:], 1e-24)
                    rec = nvec.tile([128, 1], F32, tag="nvec")
                    nc.vector.reciprocal(rec[:], n2m[:])
                    scale_c = nvec.tile([128, 1], F32, tag="nvec")
                    nc.vector.tensor_tensor(
                        scale_c[:], rec[:], temp_bc[:, c : c + 1], MULT
                    )
                    scales.append(scale_c)

                # Phase B2: s[h, n] matmuls; wws computed chunk-wise so each
                # psum accumulation group closes before the next opens
                s_ps = [ps.tile([128, 512], F32, tag="ps", name=f"s_ps{i}") for i in range(2)]
                for j in range(NCH):
                    off = 32 * (j % 4)
                    for c in range(KT):
                        wws_jc = wwsp.tile([128, CH], BF16, tag="wws")
                        nc.vector.scalar_tensor_tensor(
                            wws_jc[:],
                            wT[c][:, j * CH : (j + 1) * CH],
                            scales[c][:, 0:1],
                            wT[c][:, j * CH : (j + 1) * CH],
                            MULT,
                            MULT,
                        )
                        nc.tensor.matmul(
                            s_ps[j // 4][off : off + H, :],
                            selT[:, c, :],
                            wws_jc[:],
                            start=(c == 0),
                            stop=(c == KT - 1),
                            tile_position=(0, off),
                        )

                # Phase C: E = exp(s); Z; recipZ; rz_bc
                E_t = ep.tile([H, N], BF16, tag="E")
                for j in range(NCH):
                    off = 32 * (j % 4)
                    nc.scalar.activation(
                        E_t[:, j * CH : (j + 1) * CH],
                        s_ps[j // 4][off : off + H, :],
                        AF.Exp,
                    )
                z_ps = [ps.tile([128, 512], F32, tag="ps", name=f"z_ps{i}") for i in range(2)]
                for j in range(NCH):
                    off = 32 * (j % 4)
                    nc.tensor.matmul(
                        z_ps[j // 4][off : off + 1, :],
                        ones12[:],
                        E_t[:, j * CH : (j + 1) * CH],
                        start=True,
                        stop=True,
                        tile_position=(0, off),
                    )
                recipZ = rzvec.tile([1, N], F32, tag="recipZ")
                for j in range(NCH):
                    off = 32 * (j % 4)
                    nc.vector.reciprocal(
                        recipZ[:, j * CH : (j + 1) * CH],
                        z_ps[j // 4][off : off + 1, :],
                    )
                rz_bc = rzp.tile([128, N], F32, tag="rz")
                for j in range(NCH):
                    pst = ps.tile([128, 512], F32, tag="ps")
                    nc.tensor.matmul(
                        pst[:],
                        ones128f[:],
                        recipZ[:, j * CH : (j + 1) * CH],
                        start=True,
                        stop=True,
                    )
                    nc.scalar.copy(rz_bc[:, j * CH : (j + 1) * CH], pst[:])

                # Phase D/E: E_bc -> Pi_bc (+ per-hd sigPi accumulation)
                Pibc = []
                sig_bc = []
                for t in range(KT):
                    Pibc_t = big.tile([128, N], BF16, tag="big")
                    sig_parts = nvec.tile([128, NCH], F32, tag="nvec")
                    for j in range(NCH):
                        pst = ps.tile([128, 512], F32, tag="ps")
                        nc.tensor.matmul(
                            pst[:],
                            Bsel[:, t, :],
                            E_t[:, j * CH : (j + 1) * CH],
                            start=True,
                            stop=True,
                        )
                        nc.vector.scalar_tensor_tensor(
                            Pibc_t[:, j * CH : (j + 1) * CH],
                            pst[:],
                            1.0,
                            rz_bc[:, j * CH : (j + 1) * CH],
                            MULT,
                            MULT,
                            sig_parts[:, j : j + 1],
                        )
                    sig_t = nvec.tile([128, 1], F32, tag="nvec")
                    nc.vector.tensor_reduce(
                        sig_t[:], sig_parts[:], mybir.AxisListType.X, ADD
                    )
                    Pibc.append(Pibc_t)
                    sig_bc.append(sig_t)

                # Phase F: dots, attn; Phase G: outT (in-place over Pibc)
                for t in range(KT):
                    t3 = scr.tile([128, N], BF16, tag="scr")
                    nc.vector.tensor_tensor(t3[:], Pibc[t][:], wT[t][:], MULT)
                    junk2 = scr.tile([128, N], BF16, tag="scr")
                    dots_pre = nvec.tile([128, 1], F32, tag="nvec")
                    nc.vector.scalar_tensor_tensor(
                        junk2[:], t3[:], 1.0, wT[t][:], MULT, MULT, dots_pre[:]
                    )
                    sp = nvec.tile([128, 1], F32, tag="nvec")
                    nc.vector.tensor_scalar_add(sp[:], sig_bc[t][:], 1e-8)
                    rsp = nvec.tile([128, 1], F32, tag="nvec")
                    nc.vector.reciprocal(rsp[:], sp[:])
                    o1 = nvec.tile([128, 1], F32, tag="nvec")
                    nc.vector.tensor_scalar(
                        o1[:], dots_pre[:], rsp[:, 0:1], 1.0, MULT, ADD
                    )
                    at = nvec.tile([128, 1], F32, tag="nvec")
                    nc.vector.reciprocal(at[:], o1[:])
                    negattn = nvec.tile([128, 1], F32, tag="nvec")
                    nc.vector.tensor_scalar_mul(negattn[:], at[:], -1.0)
                    # outT_t = (Pibc_t * negattn) * wT_t, in place over Pibc_t
                    nc.vector.scalar_tensor_tensor(
                        Pibc[t][:], Pibc[t][:], negattn[:, 0:1], wT[t][:],
                        MULT, MULT,
                    )

                # Phase H: MM2 + bias, evacuate, store
                for tc_i in range(TOKT):
                    p2 = ps2.tile([128, DIM], F32, tag="ps2")
                    nc.tensor.matmul(
                        p2[:, 0:512], ones128bf[:], outb_bf[:, 0:512],
                        start=True, stop=False,
                    )
                    nc.tensor.matmul(
                        p2[:, 512:768], ones128bf[:], outb_bf[:, 512:768],
                        start=True, stop=False,
                    )
                    for k in range(KT):
                        lhs = Pibc[k][:, tc_i * 128 : (tc_i + 1) * 128]
                        nc.tensor.matmul(
                            p2[:, 0:512], lhs, ow_wT[:, k, 0:512],
                            start=False, stop=(k == KT - 1),
                        )
                        nc.tensor.matmul(
                            p2[:, 512:768], lhs, ow_wT[:, k, 512:768],
                            start=False, stop=(k == KT - 1),
                        )
                    stg = stage.tile([128, DIM], F32, tag="stage")
                    nc.scalar.copy(stg[:], p2[:])
                    nc.sync.dma_start(
                        y_d[b, tc_i * 128 : (tc_i + 1) * 128, :], stg[:]
                    )

    if split_waits:
        split_multi_waits(nc)
    nc.finalize()
    return nc


_PROGRAM = None


def _get_program():
    global _PROGRAM
    if _PROGRAM is None:
        _PROGRAM = build_program()
    return _PROGRAM


def kernel(x, qkv_w, temp, out_w, out_b):
    x = np.ascontiguousarray(np.asarray(x, dtype=np.float32))
    qkv_w = np.ascontiguousarray(np.asarray(qkv_w, dtype=np.float32))
    temp = np.ascontiguousarray(np.asarray(temp, dtype=np.float32))
    out_w = np.ascontiguousarray(np.asarray(out_w, dtype=np.float32))
    out_b = np.ascontiguousarray(np.asarray(out_b, dtype=np.float32))

    nc = _get_program()
    in_maps = [
        {
            "x": x[c * BPC : (c + 1) * BPC],
            "qkv_w": qkv_w,
            "temp": temp,
            "out_w": out_w,
            "out_b": out_b,
        }
        for c in range(N_CORES)
    ]
    res = run_bass_kernel_spmd(nc, in_maps, list(range(N_CORES)))
    return np.concatenate([r["out"] for r in res.results], axis=0)


if __name__ == "__main__":
    rng = np.random.default_rng(0)
    ins = {
        "x": rng.standard_normal((B, N, DIM)).astype(np.float32),
        "qkv_w": (rng.standard_normal((DIM, DIM)) * 0.02).astype(np.float32),
        "temp": np.ones((H, 1), np.float32),
        "out_w": (rng.standard_normal((DIM, DIM)) * 0.02).astype(np.float32),
        "out_b": np.zeros((DIM,), np.float32),
    }
    out = kernel(**ins)
    print("kernel ran, out shape", out.shape, "dtype", out.dtype)


# revision 8
# speedup vs baseline: 136.1432x; 136.1432x over previous
"""Trainium2 Bass kernel for nn_AttentionTSSA.

Contract: kernel(**inputs) takes FULL unsharded inputs (numpy), returns the
FULL output. Internally shards batch 16 -> 8 cores x 2 batches, runs one SPMD
Bass program on NeuronCores 0-7 via run_bass_kernel_spmd, and concatenates.

Pipeline (per core, per batch, "transposed" layout [hd, token]):
  x (f32, DRAM) --cast-DMA--> x_bf (bf16, DRAM) --xbar-transpose-DMA--> xT sbuf
  MM1: wT[hd, n] = qkv_wT.T @ xT           (bf16 PE, fp32 psum)
  norm2[hd] = sum_n wT^2                   (DVE tensor_tensor_reduce accum)
  scale[hd] = temp[head]/max(norm2, 1e-24)
  wws = (wT*scale)*wT                      (DVE scalar_tensor_tensor)
  s[h, n] = sum_hd sel[hd,h]*wws           (PE selector matmul, col-packed psum)
  E = exp(s)                               (ACT, no max-subtract: s ~ d/N << 1)
  Z[n] = sum_h E                           (PE ones matmul)
  rz_bc[:, n] = 1/Z[n] broadcast           (DVE recip + PE ones outer product)
  E_bc = B.T @ E  (head->hd broadcast)     (PE selector matmul)
  Pi_bc = E_bc * rz_bc, sigPi_bc = sum_n   (DVE tensor_tensor_reduce)
  dots_pre[hd] = sum_n Pi_bc*wT*wT         (DVE, 2 passes)
  negattn = -1/(1 + dots_pre/(sigPi+1e-8)) (DVE tiny)
  outT = (Pi_bc*negattn)*wT  (in placeover Pi_bc)
  MM2: y[tok, j] = outT.T @ out_wT + b     (bf16 PE, bias via K=1 ones matmul)

Works around a walrus limit (1 sync-wait per instruction) by splitting
multi-wait instructions onto inserted InstNoOp hosts.
"""

import sys

if "/opt/trn_rl_repo" not in sys.path:
    sys.path.insert(0, "/opt/trn_rl_repo")

import numpy as np
import ml_dtypes

import concourse.bass as bass
import concourse.mybir as mybir
import concourse.tile as tile
from concourse.bass_utils import run_bass_kernel_spmd

N_CORES = 8
B, N, DIM, H = 16, 4096, 768, 12
D = DIM // H          # 64
BPC = B // N_CORES    # batches per core = 2
KT = DIM // 128       # 6 k-tiles
CH = 512              # token chunk for matmuls
NCH = N // CH         # 8 chunks
TOKT = N // 128       # 32 token tiles for MM2

F32 = mybir.dt.float32
BF16 = mybir.dt.bfloat16
MULT = mybir.AluOpType.mult
ADD = mybir.AluOpType.add
AF = mybir.ActivationFunctionType

BF_NP = ml_dtypes.bfloat16


def split_multi_waits(nc, max_per_inst=1):
    """Walrus in this container rejects >1 sync wait per instruction; host
    extra waits on InstNoOp instructions inserted just before."""
    ctr = 0
    for f in nc.m.functions:
        for b in f.blocks:
            new_list, changed = [], False
            for i in b.instructions:
                si = i.sync_info
                waits = list(si.on_wait) if si and si.on_wait else []
                if len(waits) > max_per_inst:
                    extras = waits[:-max_per_inst]
                    for w in extras:
                        d = mybir.InstNoOp(name=f"waitsplit-{ctr}", ins=[], outs=[])
                        ctr += 1
                        d.engine = i.engine
                        d.sync_info = mybir.SyncInfo(on_wait=[w], on_update=[])
                        new_list.append(d)
                    si.on_wait = waits[-max_per_inst:]
                    changed = True
                new_list.append(i)
            if changed:
                b.instructions = new_list
    return ctr


def _consts():
    head = np.arange(DIM) // D  # head index per hd column
    selT = np.zeros((128, KT, H), dtype=BF_NP)
    Bsel = np.zeros((H, KT, 128), dtype=BF_NP)
    for c in range(KT):
        for p in range(128):
            h = head[c * 128 + p]
            selT[p, c, h] = 1.0
            Bsel[h, c, p] = 1.0
    ones12 = np.ones((H, 1), dtype=BF_NP)
    ones128_bf = np.ones((1, 128), dtype=BF_NP)
    ones128_f32 = np.ones((1, 128), dtype=np.float32)
    return selT, Bsel, ones12, ones128_bf, ones128_f32


def build_program(split_waits=True):
    nc = bass.Bass("TRN2", target_bir_lowering=False, debug=False)

    x_d = nc.dram_tensor("x", [BPC, N, DIM], F32, kind="ExternalInput")
    qkv_d = nc.dram_tensor("qkv_w", [DIM, DIM], F32, kind="ExternalInput")
    temp_d = nc.dram_tensor("temp", [H, 1], F32, kind="ExternalInput")
    ow_d = nc.dram_tensor("out_w", [DIM, DIM], F32, kind="ExternalInput")
    ob_d = nc.dram_tensor("out_b", [DIM], F32, kind="ExternalInput")
    y_d = nc.dram_tensor("out", [BPC, N, DIM], F32, kind="ExternalOutput")

    selT_np, Bsel_np, ones12_np, ones128bf_np, ones128f_np = _consts()
    selT_c = nc.inline_tensor(selT_np, "selT_c")
    Bsel_c = nc.inline_tensor(Bsel_np, "Bsel_c")
    ones12_c = nc.inline_tensor(ones12_np, "ones12_c")
    ones128bf_c = nc.inline_tensor(ones128bf_np, "ones128bf_c")
    ones128f_c = nc.inline_tensor(ones128f_np, "ones128f_c")

    with tile.TileContext(nc) as tc:
        with (
            tc.tile_pool(name="consts", bufs=1) as consts,
            tc.tile_pool(name="dram", bufs=1, space="DRAM") as dram,
            tc.tile_pool(name="big", bufs=6) as big,        # xT then Pibc/outT
            tc.tile_pool(name="wt", bufs=6) as wtp,
            tc.tile_pool(name="wws", bufs=4) as wwsp,
            tc.tile_pool(name="scr", bufs=2) as scr,
            tc.tile_pool(name="rz", bufs=1) as rzp,
            tc.tile_pool(name="ep", bufs=1) as ep,
            tc.tile_pool(name="nvec", bufs=10) as nvec,
            tc.tile_pool(name="rzvec", bufs=1) as rzvec,
            tc.tile_pool(name="stage", bufs=4) as stage,
            tc.tile_pool(name="ps", bufs=4, space="PSUM") as ps,
            tc.tile_pool(name="ps2", bufs=2, space="PSUM") as ps2,
        ):
            # ---- preamble: consts into SBUF ----
            selT = consts.tile([128, KT, H], BF16, tag="selT")
            nc.sync.dma_start(selT[:], selT_c[:])
            Bsel = consts.tile([H, KT, 128], BF16, tag="Bsel")
            nc.sync.dma_start(Bsel[:], Bsel_c[:])
            ones12 = consts.tile([H, 1], BF16, tag="ones12")
            nc.sync.dma_start(ones12[:], ones12_c[:])
            ones128bf = consts.tile([1, 128], BF16, tag="ones128bf")
            nc.sync.dma_start(ones128bf[:], ones128bf_c[:])
            ones128f = consts.tile([1, 128], F32, tag="ones128f")
            nc.sync.dma_start(ones128f[:], ones128f_c[:])

            outb_bf = consts.tile([1, DIM], BF16, tag="outb")
            nc.gpsimd.dma_start(
                outb_bf[:], ob_d.ap().rearrange("(a b) -> a b", a=1)
            )
            tempbf = consts.tile([H, 1], BF16, tag="tempbf")
            nc.gpsimd.dma_start(tempbf[:], temp_d[:])

            # weights: cast to bf16 in DRAM, then xbar-transpose into SBUF
            qkv_bf = dram.tile([DIM, DIM], BF16, tag="qkv_bf")
            nc.gpsimd.dma_start(qkv_bf[:], qkv_d[:])
            ow_bf = dram.tile([DIM, DIM], BF16, tag="ow_bf")
            nc.gpsimd.dma_start(ow_bf[:], ow_d[:])

            qkv_wT = consts.tile([128, KT, DIM], BF16, tag="qkv_wT")
            ow_wT = consts.tile([128, KT, DIM], BF16, tag="ow_wT")
            for k in range(KT):
                nc.sync.dma_start(
                    qkv_wT[:, k, :], qkv_bf[:, k * 128 : (k + 1) * 128],
                    transpose=True,
                )
                nc.sync.dma_start(
                    ow_wT[:, k, :], ow_bf[:, k * 128 : (k + 1) * 128],
                    transpose=True,
                )

            # temp_bc[:, c] = temp[head(hd)] for tile c (PE broadcast)
            temp_bc = consts.tile([128, KT], F32, tag="temp_bc")
            for c in range(KT):
                pst = ps.tile([128, 512], F32, tag="ps")
                nc.tensor.matmul(
                    pst[:, 0:1], Bsel[:, c, :], tempbf[:], start=True, stop=True
                )
                nc.vector.tensor_copy(temp_bc[:, c : c + 1], pst[:, 0:1])

            # x: cast both batches to bf16 DRAM up front (overlaps compute)
            x_bf = dram.tile([BPC, N, DIM], BF16, tag="x_bf")
            for b in range(BPC):
                nc.gpsimd.dma_start(x_bf[b], x_d[b])

            # ---- per-batch pipeline ----
            for b in range(BPC):
                # Phase A: xT tiles via xbar transpose (DRAM bf16 -> SBUF)
                xT = []
                for k in range(KT):
                    t = big.tile([128, N], BF16, tag="big")
                    nc.sync.dma_start(
                        t[:], x_bf[b, :, k * 128 : (k + 1) * 128], transpose=True
                    )
                    xT.append(t)

                # Phase B1: MM1 -> wT tiles; norm2/scale per tile
                wT = []
                scales = []
                for c in range(KT):
                    wT_c = wtp.tile([128, N], BF16, tag="wt")
                    for j in range(NCH):
                        pst = ps.tile([128, 512], F32, tag="ps")
                        for k in range(KT):
                            nc.tensor.matmul(
                                pst[:],
                                qkv_wT[:, k, c * 128 : (c + 1) * 128],
                                xT[k][:, j * CH : (j + 1) * CH],
                                start=(k == 0),
                                stop=(k == KT - 1),
                            )
                        nc.scalar.copy(wT_c[:, j * CH : (j + 1) * CH], pst[:])
                    wT.append(wT_c)

                    junk = scr.tile([128, N], BF16, tag="scr")
                    norm2 = nvec.tile([128, 1], F32, tag="nvec")
                    nc.vector.scalar_tensor_tensor(
                        junk[:], wT_c[:], 1.0, wT_c[:], MULT, MULT, norm2[:]
                    )
                    n2m = nvec.tile([128, 1], F32, tag="nvec")
                    nc.vector.tensor_scalar_max(n2m[:], norm2[:], 1e-24)
                    rec = nvec.tile([128, 1], F32, tag="nvec")
                    nc.vector.reciprocal(rec[:], n2m[:])
                    scale_c = nvec.tile([128, 1], F32, tag="nvec")
                    nc.vector.tensor_tensor(
                        scale_c[:], rec[:], temp_bc[:, c : c + 1], MULT
                    )
                    scales.append(scale_c)

                # Phase B2: s[h, n] matmuls; wws computed chunk-wise so each
                # psum accumulation group closes before the next opens
                s_ps = [ps.tile([128, 512], F32, tag="ps", name=f"s_ps{i}") for i in range(2)]
                for j in range(NCH):
                    off = 32 * (j % 4)
                    for c in range(KT):
                        wws_jc = wwsp.tile([128, CH], BF16, tag="wws")
                        nc.vector.scalar_tensor_tensor(
                            wws_jc[:],
                            wT[c][:, j * CH : (j + 1) * CH],
                            scales[c][:, 0:1],
                            wT[c][:, j * CH : (j + 1) * CH],
                            MULT,
                            MULT,
                        )
                        nc.tensor.matmul(
                            s_ps[j // 4][off : off + H, :],
                            selT[:, c, :],
                            wws_jc[:],
                            start=(c == 0),
                            stop=(c == KT - 1),
                            tile_position=(0, off),
                        )

                # Phase C: E = exp(s); Z; recipZ; rz_bc
                E_t = ep.tile([H, N], BF16, tag="E")
                for j in range(NCH):
                    off = 32 * (j % 4)
                    nc.scalar.activation(
                        E_t[:, j * CH : (j + 1) * CH],
                        s_ps[j // 4][off : off + H, :],
                        AF.Exp,
                    )
                z_ps = [ps.tile([128, 512], F32, tag="ps", name=f"z_ps{i}") for i in range(2)]
                for j in range(NCH):
                    off = 32 * (j % 4)
                    nc.tensor.matmul(
                        z_ps[j // 4][off : off + 1, :],
                        ones12[:],
                        E_t[:, j * CH : (j + 1) * CH],
                        start=True,
                        stop=True,
                        tile_position=(0, off),
                    )
                recipZ = rzvec.tile([1, N], F32, tag="recipZ")
                for j in range(NCH):
                    off = 32 * (j % 4)
                    nc.vector.reciprocal(
                        recipZ[:, j * CH : (j + 1) * CH],
                        z_ps[j // 4][off : off + 1, :],
                    )
                rz_bc = rzp.tile([128, N], F32, tag="rz")
                for j in range(NCH):
                    pst = ps.tile([128, 512], F32, tag="ps")
                    nc.tensor.matmul(
                        pst[:],
                        ones128f[:],
                        recipZ[:, j * CH : (j + 1) * CH],
                        start=True,
                        stop=True,
                    )
                    nc.scalar.copy(rz_bc[:, j * CH : (j + 1) * CH], pst[:])

                # Phase D/E: E_bc -> Pi_bc (+ per-hd sigPi accumulation)
                Pibc = []
                sig_bc = []
                for t in range(KT):
                    Pibc_t = big.tile([128, N], BF16, tag="big")
                    sig_parts = nvec.tile([128, NCH], F32, tag="nvec")
                    for j in range(NCH):
                        pst = ps.tile([128, 512], F32, tag="ps")
                        nc.tensor.matmul(
                            pst[:],
                            Bsel[:, t, :],
                            E_t[:, j * CH : (j + 1) * CH],
                            start=True,
                            stop=True,
                        )
                        nc.vector.scalar_tensor_tensor(
                            Pibc_t[:, j * CH : (j + 1) * CH],
                            pst[:],
                            1.0,
                            rz_bc[:, j * CH : (j + 1) * CH],
                            MULT,
                            MULT,
                            sig_parts[:, j : j + 1],
                        )
                    sig_t = nvec.tile([128, 1], F32, tag="nvec")
                    nc.vector.tensor_reduce(
                        sig_t[:], sig_parts[:], mybir.AxisListType.X, ADD
                    )
                    Pibc.append(Pibc_t)
                    sig_bc.append(sig_t)

                # Phase F: dots, attn; Phase G: outT (in-place over Pibc)
                for t in range(KT):
                    t3 = scr.tile([128, N], BF16, tag="scr")
                    nc.vector.tensor_tensor(t3[:], Pibc[t][:], wT[t][:], MULT)
                    junk2 = scr.tile([128, N], BF16, tag="scr")
                    dots_pre = nvec.tile([128, 1], F32, tag="nvec")
                    nc.vector.scalar_tensor_tensor(
                        junk2[:], t3[:], 1.0, wT[t][:], MULT, MULT, dots_pre[:]
                    )
                    sp = nvec.tile([128, 1], F32, tag="nvec")
                    nc.vector.tensor_scalar_add(sp[:], sig_bc[t][:], 1e-8)
                    rsp = nvec.tile([128, 1], F32, tag="nvec")
                    nc.vector.reciprocal(rsp[:], sp[:])
                    o1 = nvec.tile([128, 1], F32, tag="nvec")
                    nc.vector.tensor_scalar(
                        o1[:], dots_pre[:], rsp[:, 0:1], 1.0, MULT, ADD
                    )
                    at = nvec.tile([128, 1], F32, tag="nvec")
                    nc.vector.reciprocal(at[:], o1[:])
                    negattn = nvec.tile([128, 1], F32, tag="nvec")
                    nc.vector.tensor_scalar_mul(negattn[:], at[:], -1.0)
                    # outT_t = (Pibc_t * negattn) * wT_t, in place over Pibc_t
                    nc.vector.scalar_tensor_tensor(
                        Pibc[t][:], Pibc[t][:], negattn[:, 0:1], wT[t][:],
                        MULT, MULT,
                    )

                # Phase H: MM2 + bias, evacuate, store
                for tc_i in range(TOKT):
                    p2 = ps2.tile([128, DIM], F32, tag="ps2")
                    nc.tensor.matmul(
                        p2[:, 0:512], ones128bf[:], outb_bf[:, 0:512],
                        start=True, stop=False,
                    )
                    nc.tensor.matmul(
                        p2[:, 512:768], ones128bf[:], outb_bf[:, 512:768],
                        start=True, stop=False,
                    )
                    for k in range(KT):
                        lhs = Pibc[k][:, tc_i * 128 : (tc_i + 1) * 128]
                        nc.tensor.matmul(
                            p2[:, 0:512], lhs, ow_wT[:, k, 0:512],
                            start=False, stop=(k == KT - 1),
                        )
                        nc.tensor.matmul(
                            p2[:, 512:768], lhs, ow_wT[:, k, 512:768],
                            start=False, stop=(k == KT - 1),
                        )
                    stg = stage.tile([128, DIM], F32, tag="stage")
                    nc.scalar.copy(stg[:], p2[:])
                    nc.sync.dma_start(
                        y_d[b, tc_i * 128 : (tc_i + 1) * 128, :], stg[:]
                    )

    if split_waits:
        split_multi_waits(nc)
    nc.finalize()
    return nc


class _Runner:
    """Caches the Bass program, the jitted shard_map callable, and the
    per-core-replicated weights so repeat calls only move x in / out."""

    def __init__(self):
        import jax
        from jax.sharding import Mesh, PartitionSpec
        from jax.experimental.shard_map import shard_map
        from concourse import bass2jax
        import concourse.mybir as _mybir

        bass2jax.install_neuronx_cc_hook()
        self.jax = jax
        nc = build_program()
        self.nc = nc

        partition_name = (
            nc.partition_id_tensor.name if nc.partition_id_tensor else None
        )
        in_names, out_names, out_avals = [], [], []
        for alloc in nc.m.functions[0].allocations:
            if not isinstance(alloc, _mybir.MemoryLocationSet):
                continue
            name = alloc.memorylocations[0].name
            if alloc.kind == "ExternalInput":
                if name != partition_name:
                    in_names.append(name)
            elif alloc.kind == "ExternalOutput":
                out_names.append(name)
                out_avals.append(
                    jax.core.ShapedArray(
                        tuple(alloc.tensor_shape), _mybir.dt.np(alloc.dtype)
                    )
                )
        self.in_names = list(in_names)
        self.out_names = out_names
        self.out_avals = out_avals
        n_params = len(in_names)
        n_outs = len(out_names)
        all_names = in_names + out_names
        if partition_name is not None:
            all_names = all_names + [partition_name]

        def _body(*args):
            operands = list(args)
            if partition_name is not None:
                operands.append(bass2jax.partition_id_tensor())
            outs = bass2jax._bass_exec_p.bind(
                *operands,
                out_avals=tuple(out_avals),
                in_names=tuple(all_names),
                out_names=tuple(out_names),
                lowering_input_output_aliases=(),
                sim_require_finite=True,
                sim_require_nnan=True,
                nc=nc,
            )
            return tuple(outs)

        devices = jax.devices()[:N_CORES]
        self.mesh = Mesh(np.asarray(devices), ("core",))
        in_specs = (PartitionSpec("core"),) * (n_params + n_outs)
        out_specs = (PartitionSpec("core"),) * n_outs
        self.donate = tuple(range(n_params, n_params + n_outs))
        self.sharded = jax.jit(
            shard_map(
                _body,
                mesh=self.mesh,
                in_specs=in_specs,
                out_specs=out_specs,
                check_rep=False,
            ),
            donate_argnums=self.donate,
            keep_unused=True,
        )
        self.weights_dev = None

    def stage_weights(self, qkv_w, temp, out_w, out_b):
        self.weights_dev = {
            "qkv_w": self.jax.device_put(np.concatenate([qkv_w] * N_CORES, 0)),
            "temp": self.jax.device_put(np.concatenate([temp] * N_CORES, 0)),
            "out_w": self.jax.device_put(np.concatenate([out_w] * N_CORES, 0)),
            "out_b": self.jax.device_put(np.concatenate([out_b] * N_CORES, 0)),
        }

    def zeros_out(self):
        jnp = self.jax.numpy
        return [
            jnp.zeros((N_CORES * a.shape[0],) + a.shape[1:], a.dtype)
            for a in self.out_avals
        ]

    def run_raw(self, x_dev):
        """x_dev: [16, N, DIM] array (host or device). Returns device array."""
        ins = {"x": x_dev, **self.weights_dev}
        args = [ins[n] for n in self.in_names]
        outs = self.sharded(*args, *self.zeros_out())
        return outs[0]

    def __call__(self, x):
        out = self.run_raw(x)
        return np.asarray(out).reshape(B, N, DIM)


_RUNNER = None


def _get_runner():
    global _RUNNER
    if _RUNNER is None:
        _RUNNER = _Runner()
    return _RUNNER


def kernel(x, qkv_w, temp, out_w, out_b):
    x = np.ascontiguousarray(np.asarray(x, dtype=np.float32))
    qkv_w = np.ascontiguousarray(np.asarray(qkv_w, dtype=np.float32))
    temp = np.ascontiguousarray(np.asarray(temp, dtype=np.float32))
    out_w = np.ascontiguousarray(np.asarray(out_w, dtype=np.float32))
    out_b = np.ascontiguousarray(np.asarray(out_b, dtype=np.float32))

    r = _get_runner()
    r.stage_weights(qkv_w, temp, out_w, out_b)
    return r(x)


if __name__ == "__main__":
    rng = np.random.default_rng(0)
    ins = {
        "x": rng.standard_normal((B, N, DIM)).astype(np.float32),
        "qkv_w": (rng.standard_normal((DIM, DIM)) * 0.02).astype(np.float32),
        "temp": np.ones((H, 1), np.float32),
        "out_w": (rng.standard_normal((DIM, DIM)) * 0.02).astype(np.float32),
        "out_b": np.zeros((DIM,), np.float32),
    }
    out = kernel(**ins)
    print("kernel ran, out shape", out.shape, "dtype", out.dtype)
